# revision 12
# baseline (speedup 1.0000x reference)
"""Trainium2 Bass kernel for nn_Attention_82712480186499.

B=4, H=4, S=2048, D=64 attention with T5-style relative-position bias and a
key-padding mask. Returns (out [B,H,S,D], p_attn [B,H,S,S]) like the reference.

Sharding: 16 (b,h) slices over 8 cores -> 2 heads per core, both heads of a
core share the same batch b (so the mask is loaded once per core).

Per-core on-device plan (per head, all matmuls fp32 for accuracy):
  - S^T = K @ Q^T tile-by-tile on the PE, *row-packed*: key tiles are
    processed in pairs with the two [64,128] K^T stationaries living in
    partition halves 0:64 / 64:128 (contraction D=64 uses half the array, so
    two independent matmuls share it; Q^T is duplicated into both halves).
    This doubles S^T throughput vs naive fp32. Q is pre-scaled by 1/sqrt(D)
    host-side; the T5 bias (also pre-scaled) is Toeplitz so it is added from
    31 host-precomputed diagonal [128,128] tiles with one DVE add; the
    key-padding mask is a per-partition additive bias (-1e9) applied inside
    the Exp activation: E^T = exp(S^T + bias + mask) (no max subtraction
    needed: |scores| <~ 8, masked lanes underflow to exactly 0).
  - E^T tiles go straight to DRAM (the p_attn numerator, block-contiguous
    layout [g][k][512q]) and feed the PV matmul as the *moving* operand:
    out^T partial = V^T @ E^T, *col-packed*: two k-chunks' V tiles [128,64]
    are stationary in array column halves (tile_position (0,0)/(0,64)),
    accumulating even chunks into PSUM partitions 0:64 and odd chunks into
    64:128. One full-tile PSUM->SBUF copy (required: partial-partition reads
    of a col-packed accumulation race in Tile), then the halves are summed
    -> un-normalized out^T [64, 512q] per group.
  - Host computes Z = sum_k E^T (exact fp32 pairwise sum), normalizes both
    p_attn = (E^T/Z)^T and out = (out^T/Z)^T. No on-chip normalization.
"""

import math

import numpy as np

B, H, S, D = 4, 4, 2048, 64
NCORES = 8
NKT = S // 128  # 16 key tiles
NQG = S // 512  # 4 query groups
NUM_BUCKETS = 32
MAX_DISTANCE = 128

_compiled = None  # cached Bass program (compile once per process)
S_F32R = True  # experiment flag: f32r S-matmul (2x faster, ~1e-4 score err)


def _rel_bucket_vec():
    """T5 bidirectional bucket id for relative position rp = k - q, for
    rp in [-(S-1), S-1].

    Computed through the same jax ops as the reference so the f32 log /
    int-conversion semantics match the platform the reference executes on
    (the float->int32 convert truncates on XLA-CPU but rounds-to-nearest on
    the Trainium backend, and ~4% of entries sit on bucket boundaries)."""
    import jax.numpy as jnp

    rp = jnp.arange(-(S - 1), S)  # index x -> rp = x - (S-1)
    nb = NUM_BUCKETS // 2
    buckets = (rp > 0).astype(jnp.int32) * nb
    arp = jnp.abs(rp)
    max_exact = nb // 2
    is_small = arp < max_exact
    rp_f = jnp.maximum(arp, 1).astype(jnp.float32)
    rp_large = max_exact + (
        jnp.log(rp_f / max_exact)
        / math.log(MAX_DISTANCE / max_exact)
        * (nb - max_exact)
    ).astype(jnp.int32)
    rp_large = jnp.minimum(rp_large, nb - 1)
    return np.asarray(buckets + jnp.where(is_small, arp, rp_large))  # [2S-1]


def _bias_diag_tiles(bias_table):
    """[H, 31, 128, 128] f32: diag-indexed transposed-bias tiles, prescaled by
    1/sqrt(D). Position p along dim1 holds diagonal d = 15 - p; the tile for
    (key-tile tk, query-subtile tq) sits at d = tk - tq:
      tile[p_, j] = bias_table[bucket(k - q), h] / 8 with k-q = d*128 + p_ - j.
    """
    vec = _rel_bucket_vec()  # [2S-1]
    scale = 1.0 / math.sqrt(D)
    tab = np.asarray(bias_table, dtype=np.float32)  # [512, H]
    pj = np.arange(128)[:, None] - np.arange(128)[None, :]  # [128,128]
    d = 15 - np.arange(31)  # pos -> diagonal
    idx = d[:, None, None] * 128 + pj[None] + (S - 1)  # [31,128,128]
    buck = vec[idx]
    out = np.empty((H, 31, 128, 128), np.float32)
    for h in range(H):
        out[h] = tab[buck, h] * scale
    return out


def _build_bass(s_f32r=False):
    import concourse.bass as bass
    import concourse.mybir as mybir
    import concourse.tile as tile
    from concourse import bacc

    f32 = mybir.dt.float32
    f32r = mybir.dt.float32r
    nc = bacc.Bacc("TRN2", target_bir_lowering=False)

    # qt2: Q^T pre-scaled, duplicated into both partition halves [128, S]
    qt2_d = nc.declare_dram_parameter("qt2", [2, 128, S], f32, isOutput=False)
    # kt2: K^T tile pairs: pair p holds k-tile 2p in rows 0:64, 2p+1 in 64:128
    kt2_d = nc.declare_dram_parameter("kt2", [2, 128, NKT // 2, 128], f32, isOutput=False)
    v_d = nc.declare_dram_parameter("v", [2, S, D], f32, isOutput=False)
    bd_d = nc.declare_dram_parameter("bdiag", [2, 31, 128, 128], f32, isOutput=False)
    mk_d = nc.declare_dram_parameter("maskadd", [S], f32, isOutput=False)
    # E^T numerator blocks: [head][q-group][k][512 q] (tile-contiguous writes)
    et_d = nc.declare_dram_parameter("et", [2, NQG, S, 512], f32, isOutput=True)
    # un-normalized out^T per head, two partial halves stacked [128, S]
    # (even k-chunks in rows 0:64, odd in 64:128; host sums them — DVE lanes
    # are partition-locked so the cross-partition add can't run on-chip)
    ot_d = nc.declare_dram_parameter("ot", [2, 2 * D, S], f32, isOutput=True)

    EXP = mybir.ActivationFunctionType.Exp

    with tile.TileContext(nc) as tc:
        with (
            tc.tile_pool(name="const", bufs=1) as const,
            tc.tile_pool(name="perhead", bufs=2) as perhead,
            tc.tile_pool(name="epool", bufs=20) as epool,
            tc.tile_pool(name="work", bufs=4) as work,
            tc.tile_pool(name="opool", bufs=3) as opool,
            tc.tile_pool(name="pst", bufs=2, space="PSUM") as pst,
            tc.tile_pool(name="pso", bufs=2, space="PSUM") as pso,
        ):
            mask_sb = const.tile([128, NKT], f32)
            nc.sync.dma_start(
                out=mask_sb,
                in_=bass.AP(tensor=mk_d, offset=0, ap=[[1, 128], [128, NKT]]),
            )

            for hh in range(2):
                qt2 = perhead.tile([128, S], f32, tag="qt")
                kt2 = perhead.tile([128, NKT // 2, 128], f32, tag="kt")
                v_sb = perhead.tile([128, NKT, D], f32, tag="v")
                bias_sb = perhead.tile([128, 31, 128], f32, tag="bias")
                nc.sync.dma_start(out=kt2[:, 0:4, :], in_=kt2_d[hh, :, 0:4, :])
                nc.sync.dma_start(out=qt2[:, 0:512], in_=qt2_d[hh, :, 0:512])
                nc.sync.dma_start(out=kt2[:, 4:8, :], in_=kt2_d[hh, :, 4:8, :])
                for _g in range(1, NQG):
                    nc.sync.dma_start(
                        out=qt2[:, _g * 512 : (_g + 1) * 512],
                        in_=qt2_d[hh, :, _g * 512 : (_g + 1) * 512],
                    )
                nc.gpsimd.dma_start(
                    out=bias_sb,
                    in_=bass.AP(
                        tensor=bd_d,
                        offset=hh * 31 * 128 * 128,
                        ap=[[128, 128], [128 * 128, 31], [1, 128]],
                    ),
                )
                nc.gpsimd.dma_start(
                    out=v_sb,
                    in_=bass.AP(
                        tensor=v_d,
                        offset=hh * S * D,
                        ap=[[D, 128], [128 * D, NKT], [1, D]],
                    ),
                )

                if s_f32r:
                    qt2r = perhead.tile([128, S], f32r, tag="qtr")
                    kt2r = perhead.tile([128, NKT // 2, 128], f32r, tag="ktr")
                    nc.vector.tensor_copy(out=qt2r, in_=qt2)
                    nc.vector.tensor_copy(out=kt2r, in_=kt2)
                    qt_mm, kt_mm = qt2r, kt2r
                else:
                    qt_mm, kt_mm = qt2, kt2

                for g in range(NQG):
                    gs = slice(g * 512, (g + 1) * 512)
                    e_list = []
                    for p in range(NKT // 2):
                        stA = pst.tile([128, 512], f32, tag="stA")
                        stB = pst.tile([128, 512], f32, tag="stB")
                        nc.tensor.matmul(
                            out=stA, lhsT=kt_mm[0:64, p, :], rhs=qt_mm[0:64, gs],
                            start=True, stop=True,
                        )
                        nc.tensor.matmul(
                            out=stB, lhsT=kt_mm[64:128, p, :], rhs=qt_mm[64:128, gs],
                            start=True, stop=True,
                        )
                        for half, st in ((0, stA), (1, stB)):
                            t = 2 * p + half
                            pos0 = 15 - (t - 4 * g)
                            tmp = work.tile([128, 512], f32, tag="tmp")
                            nc.vector.tensor_add(
                                out=tmp,
                                in0=st,
                                in1=bias_sb[:, pos0 : pos0 + 4, :].rearrange(
                                    "p a b -> p (a b)"
                                ),
                            )
                            e = epool.tile([128, 512], f32, tag="e")
                            nc.scalar.activation(
                                out=e, in_=tmp, func=EXP,
                                bias=mask_sb[:, t : t + 1], scale=1.0,
                            )
                            e_list.append(e)
                            nc.sync.dma_start(
                                out=et_d[hh, g, t * 128 : (t + 1) * 128, :], in_=e
                            )

                    # each col-chain accumulates in its own PSUM bank so the
                    # two chains' has_written state can't interact
                    ovA = pso.tile([128, 512], f32, tag="ovA")
                    ovB = pso.tile([128, 512], f32, tag="ovB")
                    for p in range(NKT // 2):
                        nc.tensor.matmul(
                            out=ovA[0:64, :], lhsT=v_sb[:, 2 * p, :],
                            rhs=e_list[2 * p],
                            start=(p == 0), stop=(p == NKT // 2 - 1),
                            tile_position=(0, 0),
                        )
                        nc.tensor.matmul(
                            out=ovB[64:128, :], lhsT=v_sb[:, 2 * p + 1, :],
                            rhs=e_list[2 * p + 1],
                            start=(p == 0), stop=(p == NKT // 2 - 1),
                            tile_position=(0, 64),
                        )
                    # full-tile PSUM reads (partial-partition reads of a
                    # col-packed accumulation race in Tile); upper half of odA /
                    # lower half of odB are never-written garbage and unused.
                    odA = opool.tile([128, 512], f32, tag="odA")
                    odB = opool.tile([128, 512], f32, tag="odB")
                    nc.vector.tensor_copy(out=odA, in_=ovA)
                    nc.vector.tensor_copy(out=odB, in_=ovB)
                    nc.sync.dma_start(out=ot_d[hh, 0:D, gs], in_=odA[0:D, :])
                    nc.sync.dma_start(out=ot_d[hh, D : 2 * D, gs], in_=odB[D : 2 * D, :])

    nc.finalize()
    return nc


def _get_compiled():
    global _compiled
    if _compiled is None:
        _compiled = _build_bass(s_f32r=S_F32R)
    return _compiled


def kernel(query, key, value, mask, bias_table, _want_results_obj=False, _trace=False):
    from concourse.bass_utils import run_bass_kernel_spmd

    query = np.asarray(query, np.float32)
    key = np.asarray(key, np.float32)
    value = np.asarray(value, np.float32)
    mask = np.asarray(mask)
    bias_table = np.asarray(bias_table, np.float32)

    scale = 1.0 / math.sqrt(D)
    bdiag_all = _bias_diag_tiles(bias_table)  # [H, 31, 128, 128]
    maskadd_all = np.where(mask[:, 0, 0, :] == 0, np.float32(-1e9), np.float32(0.0))

    in_maps = []
    core_heads = []
    for c in range(NCORES):
        b = c // 2
        hs = [2 * (c % 2), 2 * (c % 2) + 1]
        core_heads.append((b, hs))
        qt = np.transpose(query[b, hs], (0, 2, 1)) * scale  # [2, D, S]
        qt2 = np.concatenate([qt, qt], axis=1)  # [2, 128, S]
        kt = np.transpose(key[b, hs], (0, 2, 1))  # [2, D, S]
        # pair layout: [2, 128, NKT//2, 128]
        ktt = kt.reshape(2, D, NKT // 2, 2, 128)  # [2, D, pairs, which, 128]
        kt2 = np.ascontiguousarray(
            np.concatenate([ktt[:, :, :, 0, :], ktt[:, :, :, 1, :]], axis=1)
        )
        in_maps.append(
            {
                "qt2": np.ascontiguousarray(qt2),
                "kt2": kt2,
                "v": np.ascontiguousarray(value[b, hs]),
                "bdiag": np.ascontiguousarray(bdiag_all[hs]),
                "maskadd": np.ascontiguousarray(maskadd_all[b]),
            }
        )

    nc = _get_compiled()
    res = run_bass_kernel_spmd(nc, in_maps, list(range(NCORES)), trace=_trace)

    out = np.empty((B, H, S, D), np.float32)
    p_attn = np.empty((B, H, S, S), np.float32)
    for c in range(NCORES):
        b, hs = core_heads[c]
        r = res.results[c]
        for hh, h in enumerate(hs):
            et = r["et"][hh]  # [NQG, S(k), 512(q)]
            z = et.sum(axis=1)  # [NQG, 512] softmax denominators
            for g in range(NQG):
                p_attn[b, h, g * 512 : (g + 1) * 512, :] = (et[g] / z[g]).T
            zf = z.reshape(S)  # q-major [S]
            ot = r["ot"][hh]  # [128, S]: two partial halves
            out[b, h] = ((ot[0:D] + ot[D:]) / zf[None, :]).T

    if _want_results_obj:
        return (out, p_attn), res
    return out, p_attn


# revision 13
# speedup vs baseline: 1.1631x; 1.1631x over previous
"""Trainium2 Bass kernel for nn_Attention_82712480186499.

B=4, H=4, S=2048, D=64 attention with T5-style relative-position bias and a
key-padding mask. Returns (out [B,H,S,D], p_attn [B,H,S,S]) like the reference.

Sharding: 16 (b,h) slices over 8 cores -> 2 heads per core, both heads of a
core share the same batch b (so the mask is loaded once per core).

Per-core on-device plan (per head, all matmuls fp32 for accuracy):
  - S^T = K @ Q^T tile-by-tile on the PE, *row-packed*: key tiles are
    processed in pairs with the two [64,128] K^T stationaries living in
    partition halves 0:64 / 64:128 (contraction D=64 uses half the array, so
    two independent matmuls share it; Q^T is duplicated into both halves).
    This doubles S^T throughput vs naive fp32. Q is pre-scaled by 1/sqrt(D)
    host-side; the T5 bias (also pre-scaled) is Toeplitz so it is added from
    31 host-precomputed diagonal [128,128] tiles with one DVE add; the
    key-padding mask is a per-partition additive bias (-1e9) applied inside
    the Exp activation: E^T = exp(S^T + bias + mask) (no max subtraction
    needed: |scores| <~ 8, masked lanes underflow to exactly 0).
  - E^T tiles go straight to DRAM (the p_attn numerator, block-contiguous
    layout [g][k][512q]) and feed the PV matmul as the *moving* operand:
    out^T partial = V^T @ E^T, *col-packed*: two k-chunks' V tiles [128,64]
    are stationary in array column halves (tile_position (0,0)/(0,64)),
    accumulating even chunks into PSUM partitions 0:64 and odd chunks into
    64:128. One full-tile PSUM->SBUF copy (required: partial-partition reads
    of a col-packed accumulation race in Tile), then the halves are summed
    -> un-normalized out^T [64, 512q] per group.
  - Host computes Z = sum_k E^T (exact fp32 pairwise sum), normalizes both
    p_attn = (E^T/Z)^T and out = (out^T/Z)^T. No on-chip normalization.
"""

import math

import numpy as np

B, H, S, D = 4, 4, 2048, 64
NCORES = 8
NKT = S // 128  # 16 key tiles
NQG = S // 512  # 4 query groups
NUM_BUCKETS = 32
MAX_DISTANCE = 128

_compiled = None  # cached Bass program (compile once per process)
S_F32R = False  # experiment flag: f32r S-matmul (2x faster, ~1e-4 score err)


def _rel_bucket_vec():
    """T5 bidirectional bucket id for relative position rp = k - q, for
    rp in [-(S-1), S-1].

    Computed through the same jax ops as the reference so the f32 log /
    int-conversion semantics match the platform the reference executes on
    (the float->int32 convert truncates on XLA-CPU but rounds-to-nearest on
    the Trainium backend, and ~4% of entries sit on bucket boundaries)."""
    import jax.numpy as jnp

    rp = jnp.arange(-(S - 1), S)  # index x -> rp = x - (S-1)
    nb = NUM_BUCKETS // 2
    buckets = (rp > 0).astype(jnp.int32) * nb
    arp = jnp.abs(rp)
    max_exact = nb // 2
    is_small = arp < max_exact
    rp_f = jnp.maximum(arp, 1).astype(jnp.float32)
    rp_large = max_exact + (
        jnp.log(rp_f / max_exact)
        / math.log(MAX_DISTANCE / max_exact)
        * (nb - max_exact)
    ).astype(jnp.int32)
    rp_large = jnp.minimum(rp_large, nb - 1)
    return np.asarray(buckets + jnp.where(is_small, arp, rp_large))  # [2S-1]


def _bias_diag_tiles(bias_table):
    """[H, 31, 128, 128] f32: diag-indexed transposed-bias tiles, prescaled by
    1/sqrt(D). Position p along dim1 holds diagonal d = 15 - p; the tile for
    (key-tile tk, query-subtile tq) sits at d = tk - tq:
      tile[p_, j] = bias_table[bucket(k - q), h] / 8 with k-q = d*128 + p_ - j.
    """
    vec = _rel_bucket_vec()  # [2S-1]
    scale = 1.0 / math.sqrt(D)
    tab = np.asarray(bias_table, dtype=np.float32)  # [512, H]
    pj = np.arange(128)[:, None] - np.arange(128)[None, :]  # [128,128]
    d = 15 - np.arange(31)  # pos -> diagonal
    idx = d[:, None, None] * 128 + pj[None] + (S - 1)  # [31,128,128]
    buck = vec[idx]
    out = np.empty((H, 31, 128, 128), np.float32)
    for h in range(H):
        out[h] = tab[buck, h] * scale
    return out


def _build_bass(s_f32r=False):
    import concourse.bass as bass
    import concourse.mybir as mybir
    import concourse.tile as tile
    from concourse import bacc

    f32 = mybir.dt.float32
    f32r = mybir.dt.float32r
    nc = bacc.Bacc("TRN2", target_bir_lowering=False)

    # qt2: Q^T pre-scaled, duplicated into both partition halves [128, S]
    qt2_d = nc.declare_dram_parameter("qt2", [2, 128, S], f32, isOutput=False)
    # kt2: K^T tile pairs: pair p holds k-tile 2p in rows 0:64, 2p+1 in 64:128
    kt2_d = nc.declare_dram_parameter("kt2", [2, 128, NKT // 2, 128], f32, isOutput=False)
    v_d = nc.declare_dram_parameter("v", [2, S, D], f32, isOutput=False)
    bd_d = nc.declare_dram_parameter("bdiag", [2, 31, 128, 128], f32, isOutput=False)
    mk_d = nc.declare_dram_parameter("maskadd", [S], f32, isOutput=False)
    # E^T numerator blocks: [head][q-group][k][512 q] (tile-contiguous writes)
    et_d = nc.declare_dram_parameter("et", [2, NQG, S, 512], f32, isOutput=True)
    # un-normalized out^T per head, two partial halves stacked [128, S]
    # (even k-chunks in rows 0:64, odd in 64:128; host sums them — DVE lanes
    # are partition-locked so the cross-partition add can't run on-chip)
    ot_d = nc.declare_dram_parameter("ot", [2, 2 * D, S], f32, isOutput=True)

    EXP = mybir.ActivationFunctionType.Exp

    with tile.TileContext(nc) as tc:
        with (
            tc.tile_pool(name="const", bufs=1) as const,
            tc.tile_pool(name="perhead", bufs=2) as perhead,
            tc.tile_pool(name="epool", bufs=20) as epool,
            tc.tile_pool(name="work", bufs=4) as work,
            tc.tile_pool(name="opool", bufs=3) as opool,
            tc.tile_pool(name="pst", bufs=2, space="PSUM") as pst,
            tc.tile_pool(name="pso", bufs=2, space="PSUM") as pso,
        ):
            mask_sb = const.tile([128, NKT], f32)
            nc.sync.dma_start(
                out=mask_sb,
                in_=bass.AP(tensor=mk_d, offset=0, ap=[[1, 128], [128, NKT]]),
            )

            for hh in range(2):
                qt2 = perhead.tile([128, S], f32, tag="qt")
                kt2 = perhead.tile([128, NKT // 2, 128], f32, tag="kt")
                v_sb = perhead.tile([128, NKT, D], f32, tag="v")
                bias_sb = perhead.tile([128, 31, 128], f32, tag="bias")
                nc.sync.dma_start(out=kt2[:, 0:4, :], in_=kt2_d[hh, :, 0:4, :])
                nc.sync.dma_start(out=qt2[:, 0:512], in_=qt2_d[hh, :, 0:512])
                nc.sync.dma_start(out=kt2[:, 4:8, :], in_=kt2_d[hh, :, 4:8, :])
                for _g in range(1, NQG):
                    nc.sync.dma_start(
                        out=qt2[:, _g * 512 : (_g + 1) * 512],
                        in_=qt2_d[hh, :, _g * 512 : (_g + 1) * 512],
                    )
                nc.gpsimd.dma_start(
                    out=bias_sb,
                    in_=bass.AP(
                        tensor=bd_d,
                        offset=hh * 31 * 128 * 128,
                        ap=[[128, 128], [128 * 128, 31], [1, 128]],
                    ),
                )
                nc.gpsimd.dma_start(
                    out=v_sb,
                    in_=bass.AP(
                        tensor=v_d,
                        offset=hh * S * D,
                        ap=[[D, 128], [128 * D, NKT], [1, D]],
                    ),
                )

                if s_f32r:
                    qt2r = perhead.tile([128, S], f32r, tag="qtr")
                    kt2r = perhead.tile([128, NKT // 2, 128], f32r, tag="ktr")
                    nc.vector.tensor_copy(out=qt2r, in_=qt2)
                    nc.vector.tensor_copy(out=kt2r, in_=kt2)
                    qt_mm, kt_mm = qt2r, kt2r
                else:
                    qt_mm, kt_mm = qt2, kt2

                for g in range(NQG):
                    gs = slice(g * 512, (g + 1) * 512)
                    e_list = []
                    for p in range(NKT // 2):
                        stA = pst.tile([128, 512], f32, tag="stA")
                        stB = pst.tile([128, 512], f32, tag="stB")
                        nc.tensor.matmul(
                            out=stA, lhsT=kt_mm[0:64, p, :], rhs=qt_mm[0:64, gs],
                            start=True, stop=True,
                        )
                        nc.tensor.matmul(
                            out=stB, lhsT=kt_mm[64:128, p, :], rhs=qt_mm[64:128, gs],
                            start=True, stop=True,
                        )
                        for half, st in ((0, stA), (1, stB)):
                            t = 2 * p + half
                            pos0 = 15 - (t - 4 * g)
                            tmp = work.tile([128, 512], f32, tag="tmp")
                            nc.vector.tensor_add(
                                out=tmp,
                                in0=st,
                                in1=bias_sb[:, pos0 : pos0 + 4, :].rearrange(
                                    "p a b -> p (a b)"
                                ),
                            )
                            e = epool.tile([128, 512], f32, tag="e")
                            nc.scalar.activation(
                                out=e, in_=tmp, func=EXP,
                                bias=mask_sb[:, t : t + 1], scale=1.0,
                            )
                            e_list.append(e)
                            nc.sync.dma_start(
                                out=et_d[hh, g, t * 128 : (t + 1) * 128, :], in_=e
                            )

                    # each col-chain accumulates in its own PSUM bank so the
                    # two chains' has_written state can't interact
                    ovA = pso.tile([128, 512], f32, tag="ovA")
                    ovB = pso.tile([128, 512], f32, tag="ovB")
                    for p in range(NKT // 2):
                        nc.tensor.matmul(
                            out=ovA[0:64, :], lhsT=v_sb[:, 2 * p, :],
                            rhs=e_list[2 * p],
                            start=(p == 0), stop=(p == NKT // 2 - 1),
                            tile_position=(0, 0),
                        )
                        nc.tensor.matmul(
                            out=ovB[64:128, :], lhsT=v_sb[:, 2 * p + 1, :],
                            rhs=e_list[2 * p + 1],
                            start=(p == 0), stop=(p == NKT // 2 - 1),
                            tile_position=(0, 64),
                        )
                    # full-tile PSUM reads (partial-partition reads of a
                    # col-packed accumulation race in Tile); upper half of odA /
                    # lower half of odB are never-written garbage and unused.
                    odA = opool.tile([128, 512], f32, tag="odA")
                    odB = opool.tile([128, 512], f32, tag="odB")
                    nc.vector.tensor_copy(out=odA, in_=ovA)
                    nc.vector.tensor_copy(out=odB, in_=ovB)
                    nc.sync.dma_start(out=ot_d[hh, 0:D, gs], in_=odA[0:D, :])
                    nc.sync.dma_start(out=ot_d[hh, D : 2 * D, gs], in_=odB[D : 2 * D, :])

    nc.finalize()
    return nc


def _get_compiled():
    global _compiled
    if _compiled is None:
        _compiled = _build_bass(s_f32r=S_F32R)
    return _compiled


def kernel(query, key, value, mask, bias_table, _want_results_obj=False, _trace=False):
    from concourse.bass_utils import run_bass_kernel_spmd

    query = np.asarray(query, np.float32)
    key = np.asarray(key, np.float32)
    value = np.asarray(value, np.float32)
    mask = np.asarray(mask)
    bias_table = np.asarray(bias_table, np.float32)

    scale = 1.0 / math.sqrt(D)
    bdiag_all = _bias_diag_tiles(bias_table)  # [H, 31, 128, 128]
    maskadd_all = np.where(mask[:, 0, 0, :] == 0, np.float32(-1e9), np.float32(0.0))

    in_maps = []
    core_heads = []
    for c in range(NCORES):
        b = c // 2
        hs = [2 * (c % 2), 2 * (c % 2) + 1]
        core_heads.append((b, hs))
        qt = np.transpose(query[b, hs], (0, 2, 1)) * scale  # [2, D, S]
        qt2 = np.concatenate([qt, qt], axis=1)  # [2, 128, S]
        kt = np.transpose(key[b, hs], (0, 2, 1))  # [2, D, S]
        # pair layout: [2, 128, NKT//2, 128]
        ktt = kt.reshape(2, D, NKT // 2, 2, 128)  # [2, D, pairs, which, 128]
        kt2 = np.ascontiguousarray(
            np.concatenate([ktt[:, :, :, 0, :], ktt[:, :, :, 1, :]], axis=1)
        )
        in_maps.append(
            {
                "qt2": np.ascontiguousarray(qt2),
                "kt2": kt2,
                "v": np.ascontiguousarray(value[b, hs]),
                "bdiag": np.ascontiguousarray(bdiag_all[hs]),
                "maskadd": np.ascontiguousarray(maskadd_all[b]),
            }
        )

    nc = _get_compiled()
    res = run_bass_kernel_spmd(nc, in_maps, list(range(NCORES)), trace=_trace)

    out = np.empty((B, H, S, D), np.float32)
    p_attn = np.empty((B, H, S, S), np.float32)
    for c in range(NCORES):
        b, hs = core_heads[c]
        r = res.results[c]
        for hh, h in enumerate(hs):
            et = r["et"][hh]  # [NQG, S(k), 512(q)]
            z = et.sum(axis=1)  # [NQG, 512] softmax denominators
            for g in range(NQG):
                p_attn[b, h, g * 512 : (g + 1) * 512, :] = (et[g] / z[g]).T
            zf = z.reshape(S)  # q-major [S]
            ot = r["ot"][hh]  # [128, S]: two partial halves
            out[b, h] = ((ot[0:D] + ot[D:]) / zf[None, :]).T

    if _want_results_obj:
        return (out, p_attn), res
    return out, p_attn
